# revision 1
# baseline (speedup 1.0000x reference)
"""Trainium2 Bass kernel for EquivariantMultiheadAttention (sparse attention).

Problem shapes: b=4, n=512, c=256, h=8, d=32, dg=6, hid=16.

Strategy (8 NeuronCores, no collectives):
  - Shard (batch b x n-half): core i handles b = i//2, query rows
    n0 = 256*(i%2) .. n0+256.  Each core computes its 256 output rows
    fully; keys/values are replicated per batch.
  - Sparse-attention compaction: keys with mask[b,m]=False contribute
    exactly 0 (softmax of -1e38), so they are dropped on the host.  Keys
    are gathered to a compact, zero-padded m_pad (>=256, mult of 16);
    padded columns get a -1e38 bias so they vanish in the softmax.  This
    is exact and roughly halves all device work.
  - Host pre-transposes pairwise_g to [6, n', m_pad] so the per-head
    location-MLP layer-1 matmul needs no on-device transpose.
  - MLP runs as chained PE matmuls (all heads packed: layer1 [6->128],
    layer2 block-diag [128->128], layer3 [128->8] via 16 zero-padded
    full-width W3 variants accumulated in PSUM) with ACT Silu reading
    PSUM directly (bias fused).  float32r (TF32-like, ~2e-4 rel err)
    gives 4x PE throughput on the hot matmuls.
  - P1 is software-pipelined: per step i, PE runs [l1(i) | l2(i-1) |
    l3(i-2)] while ACT runs [silu1(i), silu2(i-1)] back-to-back; the
    prologue (Q/K/V projections, A_feat groups, big const DMAs) is
    interleaved into the pipeline so ACT never waits on it.
  - A_feat = Q K^T / sqrt(c) accumulates into the same [16 rows x 8
    heads, m_pad] PSUM layout via a masked-Q lhsT plus the -1e38 pad
    bias row, so `pre` needs one DVE add.
  - Softmax: DVE max (negated) -> ACT Exp with bias=-max and fused row
    sums -> DVE reciprocal + scale.  All Silu ops are emitted before all
    Exp ops so the ACT table set switches exactly once.
  - att is transposed per 128-col slice on the PE, then AV runs in f16
    (full-rate, 32-aligned dst allowed) and the output projection in
    f32r.
"""
import sys

sys.path.insert(0, "/opt/trn_rl_repo")

import numpy as np
import concourse.bacc as bacc
import concourse.mybir as mybir
import concourse.tile as tile
from concourse.bass_utils import run_bass_kernel_spmd

F32 = mybir.dt.float32
F32R = mybir.dt.float32r
F16 = mybir.dt.float16
AF = mybir.ActivationFunctionType
AX = mybir.AxisListType
ALU = mybir.AluOpType

B, N, C, H, DG, HID = 4, 512, 256, 8, 6, 16
D = C // H          # 32
NP = N // 2         # 256 query rows per core
NG = NP // 16       # 16 groups of 16 rows
NEG = np.float32(-1e38)
AV_DT = F16         # dtype for the att @ V contraction


def _build(nc_mod, m_pad):
    """Emit the SPMD single-core program. m_pad: compacted+padded key count."""
    nc = nc_mod
    M = m_pad
    MT = [(t, min(128, M - 128 * t)) for t in range((M + 127) // 128)]

    # ---------------- I/O ----------------
    gt = nc.declare_dram_parameter("gt", [DG, NP, M], F32, isOutput=False)
    ctq = nc.declare_dram_parameter("ctq", [C, NP], F32, isOutput=False)
    ctk = nc.declare_dram_parameter("ctk", [C, M], F32, isOutput=False)
    wq = nc.declare_dram_parameter("wq", [C, C], F32, isOutput=False)
    wk = nc.declare_dram_parameter("wk", [C, C], F32, isOutput=False)
    win = nc.declare_dram_parameter("win", [C, C], F32, isOutput=False)
    wout = nc.declare_dram_parameter("wout", [C, C], F32, isOutput=False)
    bq = nc.declare_dram_parameter("bq", [1, C], F32, isOutput=False)
    bk = nc.declare_dram_parameter("bk", [1, C], F32, isOutput=False)
    bin_ = nc.declare_dram_parameter("bin", [1, C], F32, isOutput=False)
    bout = nc.declare_dram_parameter("bout", [1, C], F32, isOutput=False)
    w1 = nc.declare_dram_parameter("w1", [DG, 128], F32, isOutput=False)
    w2 = nc.declare_dram_parameter("w2", [128, 128], F32, isOutput=False)
    w3 = nc.declare_dram_parameter("w3", [128, 8], F32, isOutput=False)
    b1 = nc.declare_dram_parameter("b1", [128, 1], F32, isOutput=False)
    b2 = nc.declare_dram_parameter("b2", [128, 1], F32, isOutput=False)
    b3 = nc.declare_dram_parameter("b3", [128, 1], F32, isOutput=False)
    mb = nc.declare_dram_parameter("mb", [1, M], F32, isOutput=False)
    onesc = nc.declare_dram_parameter("onesc", [1, 128], F32, isOutput=False)
    zeros = nc.declare_dram_parameter("zeros", [128, 128], F32, isOutput=False)
    ident = nc.declare_dram_parameter("ident", [128, 128], F32, isOutput=False)
    out = nc.declare_dram_parameter("out", [NP, C], F32, isOutput=True)

    with tile.TileContext(nc) as tc:
        import contextlib
        with contextlib.ExitStack() as ctx:
            cst = ctx.enter_context(tc.tile_pool(name="cst", bufs=1))
            big = ctx.enter_context(tc.tile_pool(name="big", bufs=1))
            gtp = ctx.enter_context(tc.tile_pool(name="gtp", bufs=3))
            xp = ctx.enter_context(tc.tile_pool(name="xp", bufs=3))
            smp = ctx.enter_context(tc.tile_pool(name="smp", bufs=3))
            pmm = ctx.enter_context(tc.tile_pool(name="pmm", bufs=3, space="PSUM"))
            pl3 = ctx.enter_context(tc.tile_pool(name="pl3", bufs=1, space="PSUM"))
            pms = ctx.enter_context(tc.tile_pool(name="pms", bufs=1, space="PSUM"))

            # ---- critical-path constants first: w1/w2/biases ----
            w1_sb = cst.tile([DG, 128], F32R, tag="w1")
            nc.gpsimd.dma_start(out=w1_sb, in_=w1[:, :].bitcast(F32R))
            b1_sb = cst.tile([128, 1], F32, tag="b1")
            b2_sb = cst.tile([128, 1], F32, tag="b2")
            b3_sb = cst.tile([128, 1], F32, tag="b3")
            nc.gpsimd.dma_start(out=b1_sb, in_=b1[:, :])
            nc.gpsimd.dma_start(out=b2_sb, in_=b2[:, :])
            nc.gpsimd.dma_start(out=b3_sb, in_=b3[:, :])
            w2_sb = cst.tile([128, 128], F32R, tag="w2")
            nc.gpsimd.dma_start(out=w2_sb, in_=w2[:, :].bitcast(F32R))

            # ---- deferred-prologue emitters (interleaved into pipeline) ----
            w3_sb = [cst.tile([128, 128], F32R, tag=f"w3{j}", name=f"w3{j}")
                     for j in range(16)]
            qt_sb = [big.tile([128, NP], F32R, tag=f"qt{i}", name=f"qt{i}")
                     for i in range(2)]
            kt_sb = [big.tile([128, M], F32R, tag=f"kt{i}", name=f"kt{i}")
                     for i in range(2)]
            v_sb = [big.tile([128, C], AV_DT, tag=f"v{t}", name=f"v{t}")
                    for t, _ in MT]
            wq_sb, wk_sb, win_sb, wout_sb = [], [], [], []
            ctq_sb, ctk_sb = [], []
            misc = {}

            def emit_w3_dmas():
                # variants are 99% zeros: build with DVE copies instead of
                # shipping 1MB over the (serialized) DMA queue
                zs = cst.tile([128, 128], F32R, tag="zs", name="zs")
                nc.gpsimd.dma_start(out=zs, in_=zeros[:, :].bitcast(F32R))
                w3c = cst.tile([128, 8], F32R, tag="w3c", name="w3c")
                nc.gpsimd.dma_start(out=w3c, in_=w3[:, :].bitcast(F32R))
                for j in range(16):
                    nc.vector.tensor_copy(out=w3_sb[j], in_=zs)
                    nc.vector.tensor_copy(out=w3_sb[j][:, 8 * j:8 * j + 8],
                                          in_=w3c)
                emit_qm_zero(zs)

            def emit_qk_consts():
                for ci in range(2):
                    t = cst.tile([128, C], F32, tag=f"wq{ci}", name=f"wq{ci}")
                    nc.gpsimd.dma_start(out=t, in_=wq[128 * ci:128 * (ci + 1), :])
                    wq_sb.append(t)
                    t = cst.tile([128, C], F32, tag=f"wk{ci}", name=f"wk{ci}")
                    nc.gpsimd.dma_start(out=t, in_=wk[128 * ci:128 * (ci + 1), :])
                    wk_sb.append(t)
                    t = cst.tile([128, NP], F32, tag=f"cq{ci}", name=f"cq{ci}")
                    nc.gpsimd.dma_start(out=t, in_=ctq[128 * ci:128 * (ci + 1), :])
                    ctq_sb.append(t)
                    t = cst.tile([128, M], F32, tag=f"ck{ci}", name=f"ck{ci}")
                    nc.gpsimd.dma_start(out=t, in_=ctk[128 * ci:128 * (ci + 1), :])
                    ctk_sb.append(t)
                for nm, src in (("bq", bq), ("bk", bk)):
                    t = cst.tile([1, C], F32, tag=nm, name=nm)
                    nc.gpsimd.dma_start(out=t, in_=src[:, :])
                    misc[nm] = t
                t = cst.tile([1, 512], F32, tag="onr", name="onr")
                nc.vector.memset(t, 1.0)
                misc["onr"] = t
                t = cst.tile([1, M], F32R, tag="mb", name="mbt")
                nc.gpsimd.dma_start(out=t, in_=mb[:, :].bitcast(F32R))
                misc["mb"] = t
                t = cst.tile([1, 128], F32R, tag="onc", name="onct")
                nc.gpsimd.dma_start(out=t, in_=onesc[:, :].bitcast(F32R))
                misc["onc"] = t

            def proj_T(dst, w_tiles, b_row, rhs_tiles, nfree):
                for ct in range(2):
                    p = pms.tile([128, 512], F32, tag="ms",
                                 name=f"pj{ct}_{len(wq_sb)}_{len(win_sb)}")
                    for ci in range(2):
                        nc.tensor.matmul(
                            p[:, :nfree],
                            w_tiles[ci][:, 128 * ct:128 * (ct + 1)],
                            rhs_tiles[ci][:, :nfree],
                            start=(ci == 0), stop=False)
                    nc.tensor.matmul(
                        p[:, :nfree], b_row[:, 128 * ct:128 * (ct + 1)],
                        misc["onr"][:, :nfree], start=False, stop=True)
                    nc.vector.tensor_copy(out=dst[ct], in_=p[:, :nfree])

            def emit_qt():
                proj_T(qt_sb, wq_sb, misc["bq"], ctq_sb, NP)

            def emit_kt():
                proj_T(kt_sb, wk_sb, misc["bk"], ctk_sb, M)

            def emit_v_consts():
                for ci in range(2):
                    t = cst.tile([128, C], F32, tag=f"wi{ci}", name=f"wi{ci}")
                    nc.gpsimd.dma_start(out=t, in_=win[128 * ci:128 * (ci + 1), :])
                    win_sb.append(t)
                t = cst.tile([1, C], F32, tag="bi", name="bi")
                nc.gpsimd.dma_start(out=t, in_=bin_[:, :])
                misc["bi"] = t

            def emit_v():
                for t_, msz in MT:
                    p = pms.tile([128, 512], F32, tag="ms", name=f"pv{t_}")
                    for ci in range(2):
                        nc.tensor.matmul(
                            p[:msz, :C],
                            ctk_sb[ci][:, 128 * t_:128 * t_ + msz],
                            win_sb[ci], start=(ci == 0), stop=False)
                    nc.tensor.matmul(
                        p[:msz, :C], misc["onc"][:, :msz].bitcast(F32),
                        misc["bi"], start=False, stop=True)
                    nc.vector.tensor_copy(out=v_sb[t_][:msz, :],
                                          in_=p[:msz, :C])

            def emit_tail_consts():
                t = cst.tile([128, 128], F32R, tag="id", name="idt")
                nc.gpsimd.dma_start(out=t, in_=ident[:, :].bitcast(F32R))
                misc["id"] = t
                for ci in range(2):
                    t = cst.tile([128, C], F32R, tag=f"wo{ci}", name=f"wo{ci}")
                    nc.gpsimd.dma_start(
                        out=t,
                        in_=wout[128 * ci:128 * (ci + 1), :].bitcast(F32R))
                    wout_sb.append(t)
                t = cst.tile([1, C], F32, tag="bo", name="bo")
                nc.gpsimd.dma_start(out=t, in_=bout[:, :])
                misc["bo"] = t

            # ------------- P1: A_feat + location MLP, software-pipelined ----
            af_sb = big.tile([128, NG, M], F32, tag="af")
            qmp = [
                [big.tile([128, 16, 8], F32R, tag=f"qm{p}{i}", name=f"qm{p}{i}")
                 for i in range(2)]
                for p in range(2)
            ]
            # zero-filled via DVE in emit_qm_zero (after zs loads)

            pre_all = big.tile([128, NG, M], F32, tag="pre")
            mxn_all = big.tile([128, NG], F32, tag="mxn")

            SB = [(g, [2 * t + j for j in range(2) if 2 * t + j < 16])
                  for g in range(NG) for t in range(8)]
            NSB = len(SB)
            gt_tiles = {}
            x1s, x2s = {}, {}
            p3_cur = [None]

            def emit_qm_zero(zs_tile):
                for p_ in range(2):
                    for i_ in range(2):
                        nc.vector.tensor_copy(
                            out=qmp[p_][i_],
                            in_=zs_tile.rearrange("p (a b) -> p a b", a=16))

            def emit_af(g):
                qm = qmp[g % 2]
                for h in range(H):
                    a = 32 * (h % 4)
                    nc.vector.tensor_copy(
                        out=qm[h // 4][a:a + 32, :, h],
                        in_=qt_sb[h // 4][a:a + 32, 16 * g:16 * g + 16])
                paf = pms.tile([128, 512], F32, tag="ms", name=f"paf{g}")
                nc.tensor.matmul(paf[:, :M], qm[0], kt_sb[0],
                                 start=True, stop=False)
                nc.tensor.matmul(paf[:, :M], qm[1], kt_sb[1],
                                 start=False, stop=False)
                nc.tensor.matmul(paf[:, :M], misc["onc"], misc["mb"],
                                 start=False, stop=True)
                nc.vector.tensor_copy(out=af_sb[:, g, :], in_=paf[:, :M])

            # deferred prologue work keyed by pipeline step
            deferred = {
                0: [emit_qk_consts],
                1: [emit_w3_dmas],
                3: [emit_qt],
                4: [emit_kt],
                5: [emit_v_consts],
                6: [emit_v],
                7: [emit_tail_consts],
            }

            for i in range(NSB + 2):
                for fn in deferred.get(i, ()):
                    fn()
                if i < NSB:
                    g, rows = SB[i]
                    nr = len(rows)
                    if rows[0] == 0:
                        gt_t = gtp.tile([DG, 16, M], F32R, tag="gt",
                                        name=f"gt{g}")
                        nc.sync.dma_start(
                            out=gt_t,
                            in_=gt[:, 16 * g:16 * (g + 1), :].bitcast(F32R))
                        gt_tiles[g] = gt_t
                    p1t = pmm.tile([128, 2, 512], F32, tag="mm", name=f"p1_{i}")
                    for j, r in enumerate(rows):
                        nc.tensor.matmul(p1t[:, j, :M], w1_sb,
                                         gt_tiles[g][:, r, :],
                                         start=True, stop=True)
                    x1 = xp.tile([128, 2, M], F32R, tag="x1", name=f"x1_{i}")
                    nc.scalar.activation(out=x1[:, :nr, :], in_=p1t[:, :nr, :M],
                                         func=AF.Silu, bias=b1_sb, scale=1.0)
                    x1s[i] = x1
                if 0 <= i - 1 < NSB:
                    k = i - 1
                    g, rows = SB[k]
                    nr = len(rows)
                    p2t = pmm.tile([128, 2, 512], F32, tag="mm", name=f"p2_{k}")
                    for j in range(nr):
                        nc.tensor.matmul(p2t[:, j, :M], w2_sb, x1s[k][:, j, :],
                                         start=True, stop=True)
                    x2 = xp.tile([128, 2, M], F32R, tag="x2", name=f"x2_{k}",
                                 bufs=6)
                    nc.scalar.activation(out=x2[:, :nr, :], in_=p2t[:, :nr, :M],
                                         func=AF.Silu, bias=b2_sb, scale=1.0)
                    x2s[k] = x2
                    del x1s[k]
                if 0 <= i - 2 < NSB:
                    k = i - 2
                    g, rows = SB[k]
                    if rows[0] == 0:
                        p3_cur[0] = pl3.tile([128, 512], F32, tag="l3",
                                             name=f"p3_{g}")
                    if rows[0] == 8:
                        # mid-group: QT/KT (and their DMAs) are ready by now,
                        # and af(g) is only consumed at the end of group g
                        emit_af(g)
                    p3 = p3_cur[0]
                    for j, r in enumerate(rows):
                        # f32r requires dst partition 0: each row uses a
                        # zero-padded full-width W3 variant, all 16 rows
                        # accumulate into the full [128, M] region
                        nc.tensor.matmul(p3[:, :M], w3_sb[r], x2s[k][:, j, :],
                                         start=(r == 0), stop=(r == 15))
                    if rows[-1] == 15:
                        aloc = smp.tile([128, M], F32, tag="aloc",
                                        name=f"aloc{g}")
                        nc.scalar.activation(out=aloc, in_=p3[:, :M],
                                             func=AF.Silu, bias=b3_sb,
                                             scale=1.0)
                        nc.vector.tensor_add(pre_all[:, g, :], aloc,
                                             af_sb[:, g, :])
                        nc.vector.tensor_reduce(
                            out=mxn_all[:, g:g + 1], in_=pre_all[:, g, :],
                            axis=AX.X, op=ALU.max, negate=True)
                    del x2s[k]

            # ------------- P2: exp softmax + transpose, all groups -------
            attT = big.tile([128, len(MT), 16, 16, 8], AV_DT, tag="attT")
            for g in range(NG):
                att = smp.tile([128, M], F32, tag="att", name=f"att{g}")
                sm = smp.tile([128, 1], F32, tag="sm", name=f"sm{g}")
                nc.scalar.activation(out=att, in_=pre_all[:, g, :], func=AF.Exp,
                                     bias=mxn_all[:, g:g + 1], scale=1.0,
                                     accum_out=sm)
                rc = smp.tile([128, 1], F32, tag="rc", name=f"rc{g}")
                nc.vector.reciprocal(out=rc, in_=sm)
                atts = smp.tile([128, M], F32R, tag="atts", name=f"atts{g}")
                nc.vector.tensor_scalar_mul(atts, att, rc)
                pT = (pms if g % 2 == 0 else pl3).tile(
                    [128, 512], F32R, tag=("ms" if g % 2 == 0 else "l3"),
                    name=f"pT{g}")
                for t, msz in MT:
                    nc.tensor.transpose(
                        pT[:msz, 128 * t:128 * t + 128],
                        atts[:, 128 * t:128 * t + msz],
                        misc["id"])
                    nc.vector.tensor_copy(
                        out=attT[:msz, t, g, :, :],
                        in_=pT[:msz, 128 * t:128 * t + 128])

            # ------------- P3: AV (f16) + output projection -------------
            pavs = []
            for t, _ in MT:
                if t % 3 == 2:
                    pavs.append(pms.tile([128, 2, 256], F32, tag="ms",
                                         name=f"pav{t}"))
                else:
                    pt_ = pmm.tile([128, 2, 512], F32, tag="mm",
                                   name=f"pav{t}")
                    pavs.append(pt_[:, 0, :].rearrange("p (a b) -> p a b", a=2))
            for h in range(H):
                for t, msz in MT:
                    nc.tensor.matmul(
                        pavs[t][32 * (h % 4):32 * (h % 4) + 32, h // 4, :],
                        v_sb[t][:msz, 32 * h:32 * h + 32],
                        attT[:msz, t, :, :, h],
                        start=True, stop=True,
                        tile_position=(0, 32 * (h % 4)))
            avf = smp.tile([128, 2, 256], F32R, tag="avf")
            if len(MT) == 1:
                nc.vector.tensor_copy(out=avf, in_=pavs[0])
            else:
                acc = smp.tile([128, 2, 256], F32, tag="avacc")
                nc.vector.tensor_copy(out=acc, in_=pavs[0])
                for t in range(1, len(MT) - 1):
                    nc.vector.tensor_add(acc, acc, pavs[t])
                nc.vector.tensor_add(avf, acc, pavs[len(MT) - 1])
            avT = [avf[:, i, :] for i in range(2)]
            for nt in range(2):
                po = pl3.tile([128, 512], F32, tag="l3", name=f"po{nt}")
                for ci in range(2):
                    nc.tensor.matmul(
                        po[:, :C], avT[ci][:, 128 * nt:128 * (nt + 1)],
                        wout_sb[ci], start=(ci == 0), stop=False)
                nc.tensor.matmul(po[:, :C], misc["onc"].bitcast(F32),
                                 misc["bo"], start=False, stop=True)
                o_sb = smp.tile([128, C], F32, tag="osb", name=f"osb{nt}")
                nc.vector.tensor_copy(out=o_sb, in_=po[:, :C])
                nc.sync.dma_start(out=out[128 * nt:128 * (nt + 1), :], in_=o_sb)

    nc.finalize()
    return nc


_CACHE = {}


def _get_nc(m_pad):
    if m_pad not in _CACHE:
        _CACHE[m_pad] = _build(bacc.Bacc(None, target_bir_lowering=False), m_pad)
    return _CACHE[m_pad]


def prepare(inputs):
    """Host-side sharding/packing. Returns (nc, in_maps, assemble)."""
    pg = np.asarray(inputs["pairwise_g"], np.float32)
    cf = np.asarray(inputs["coset_functions"], np.float32)
    mask = np.asarray(inputs["mask"])
    idxs = [np.where(mask[b])[0] for b in range(B)]
    maxc = max(len(ix) for ix in idxs)
    m_pad = max(256, -(-maxc // 16) * 16)

    w1a = np.ascontiguousarray(
        np.asarray(inputs["loc_w1"], np.float32).transpose(1, 0, 2).reshape(
            DG, 128))
    w2b = np.zeros((128, 128), np.float32)
    lw2 = np.asarray(inputs["loc_w2"], np.float32)
    for h in range(H):
        w2b[16 * h:16 * (h + 1), 16 * h:16 * (h + 1)] = lw2[h]
    lw3 = np.asarray(inputs["loc_w3"], np.float32)
    w3p = np.zeros((128, 8), np.float32)
    for h in range(H):
        w3p[16 * h:16 * (h + 1), h] = lw3[h, :, 0]
    b1v = np.asarray(inputs["loc_b1"], np.float32).reshape(128, 1)
    b2v = np.asarray(inputs["loc_b2"], np.float32).reshape(128, 1)
    b3v = np.tile(np.asarray(inputs["loc_b3"], np.float32).reshape(8), 16)
    b3v = b3v.reshape(128, 1)

    common = {
        "wq": np.asarray(inputs["fc_q_w"], np.float32) / np.float32(16.0),
        "wk": np.asarray(inputs["fc_k_w"], np.float32),
        "win": np.asarray(inputs["in_w"], np.float32),
        "wout": np.asarray(inputs["out_w"], np.float32),
        "bq": (np.asarray(inputs["fc_q_b"], np.float32) / np.float32(16.0)
               ).reshape(1, C),
        "bk": np.asarray(inputs["fc_k_b"], np.float32).reshape(1, C),
        "bin": np.asarray(inputs["in_b"], np.float32).reshape(1, C),
        "bout": np.asarray(inputs["out_b"], np.float32).reshape(1, C),
        "w1": w1a, "w2": w2b, "w3": w3p,
        "b1": b1v, "b2": b2v, "b3": b3v,
        "ident": np.eye(128, dtype=np.float32),
        "onesc": np.ones((1, 128), np.float32),
        "zeros": np.zeros((128, 128), np.float32),
    }
    common = {k: np.ascontiguousarray(v) for k, v in common.items()}

    in_maps = []
    for core in range(8):
        b, nh = core // 2, core % 2
        ix = idxs[b]
        cnt = len(ix)
        n0 = NP * nh
        gtc = np.zeros((DG, NP, m_pad), np.float32)
        gtc[:, :, :cnt] = pg[b, n0:n0 + NP][:, ix, :].transpose(2, 0, 1)
        ctk = np.zeros((C, m_pad), np.float32)
        ctk[:, :cnt] = cf[b, ix, :].T
        mbv = np.zeros((1, m_pad), np.float32)
        mbv[0, cnt:] = NEG
        im = dict(common)
        im["gt"] = np.ascontiguousarray(gtc)
        im["ctq"] = np.ascontiguousarray(cf[b, n0:n0 + NP, :].T)
        im["ctk"] = np.ascontiguousarray(ctk)
        im["mb"] = mbv
        in_maps.append(im)

    def assemble(results):
        o = np.empty((B, N, C), np.float32)
        for core in range(8):
            b, nh = core // 2, core % 2
            o[b, NP * nh:NP * (nh + 1), :] = results[core]["out"]
        return o

    return _get_nc(m_pad), in_maps, assemble


def kernel(**inputs) -> np.ndarray:
    nc, in_maps, assemble = prepare(inputs)
    res = run_bass_kernel_spmd(nc, in_maps, list(range(8)))
    return assemble(res.results)

